# revision 25
# baseline (speedup 1.0000x reference)
"""MALA attention (linear attention w/ 2D RoPE + magnitude term) on 8 trn2 cores.

Sharding: core i handles batch b = i//2, sequence rows (i%2)*2048..+2048.
Cross-core data (kv = k_rope^T v, k_sum, v_sum -- all sums over the full
sequence) is combined with a pairwise AllReduce (~210KB). Everything else is
local. bf16 operands everywhere with fp32 PSUM accumulation.

Math (per batch b, head h, reference semantics):
  q = elu(query @ Wq.T + bq) + 1      (same k with Wk/bk; v plain)
  z = (q . mean_n(k)) * d^-0.5
  q, k <- rope(q), rope(k)
  kv = k^T v * (d^-0.5 / N)
  res = (q @ kv) * (1 + 1/(z+1e-6)) - z * mean_n(v)
  out = res @ Wo.T + bo

Device-side formulation:
  elu(x)+1 = max(x + 1, min(exp(x), 1))    [exact: exp(min(x,0)) = min(exp x,1)]
  k/v bias (+1 for k) added via DVE from pre-broadcast [128,C] bias tiles;
  q bias via per-partition Act bias (features on partitions there). All
  bias-opener matmuls eliminated.
  g = 1 + 1/(z+eps) folded into q_rope before q@kv (G built by one-hot matmul).
  -z*v_mean term folded into o_proj as an extra matmul (-W~)^T @ z where
  W~[c,h] = Wo[:, head h] @ v_mean_h.
  The within-head d-index is permuted (evens first) on Wq/Wk columns + trig
  tables so rotate_every_two becomes a 32-column block swap.
  Collective payload is slimmed to kv v-halves+vsum row [65,768] + per-head
  ksums pre-transposed to columns [128,12] (12 tiny K=1 matmuls; vsum
  transposes happen post-collective, off the trigger path), so the
  post-collective critical path is 2 DMAs + 2 strided Act copies.
"""

import os
import tempfile

import numpy as np
import ml_dtypes

NUM_HEADS = 12
B, N, C = 4, 4096, 768
D = 64
NCORES = 8
NLOC = N // 2          # rows per core
SCALE = D ** -0.5
BF16 = ml_dtypes.bfloat16

_CACHE = {}
LAST_RESULTS = [None]  # test.py reads profiling info from here


# --------------------------------------------------------------------------
# host-side helpers
# --------------------------------------------------------------------------

def _perm64():
    # evens first, then odds (within each head's 64 dims)
    return np.concatenate([np.arange(0, 64, 2), np.arange(1, 64, 2)])


def _trig_tables():
    """c32/s32: [N, 32] fp32, value of cos/sin at original dim 2i (== 2i+1)."""
    H = W = 64
    angle = 1.0 / (10000.0 ** np.linspace(0.0, 1.0, D // 4))
    angle = np.repeat(angle, 2)                          # [32]
    ih = np.arange(H, dtype=np.float64)[:, None] * angle[None, :]   # [H, 32]
    iw = np.arange(W, dtype=np.float64)[:, None] * angle[None, :]
    sin_h, cos_h = np.sin(ih), np.cos(ih)
    sin_w, cos_w = np.sin(iw), np.cos(iw)
    r = np.arange(N) // W
    c = np.arange(N) % W
    s_full = np.concatenate([sin_h[r], sin_w[c]], axis=1)   # [N, 64]
    c_full = np.concatenate([cos_h[r], cos_w[c]], axis=1)
    c32 = c_full[:, 0::2].astype(np.float32)
    s32 = s_full[:, 0::2].astype(np.float32)
    return c32, s32


def _build_host_inputs(query, key, value, Wq, bq, Wk, bk, Wv, bv, Wo, bo):
    p64 = _perm64()
    perm = (np.arange(NUM_HEADS)[:, None] * 64 + p64[None, :]).reshape(-1)

    wq = np.ascontiguousarray(Wq.T[:, perm]).astype(BF16)
    wk = np.ascontiguousarray(Wk.T[:, perm]).astype(BF16)
    wv = np.ascontiguousarray(Wv.T).astype(BF16)
    wo = np.ascontiguousarray(Wo.T).astype(BF16)
    bqp1 = (bq[perm] + 1.0).astype(BF16)
    # k/v biases pre-broadcast across partitions for the DVE bias add
    bkp1_b = np.tile((bk[perm] + 1.0).astype(BF16)[None, :], (128, 1))
    bv_b = np.tile(bv.astype(BF16)[None, :], (128, 1))
    bof = bo.astype(np.float32)

    c32, s32 = _trig_tables()
    halves = []
    for hi in range(2):
        sl = slice(hi * NLOC, (hi + 1) * NLOC)
        ck = np.concatenate([c32[sl], c32[sl]], axis=1).astype(BF16)    # [NLOC, 64]
        s2k = np.concatenate([-s32[sl], s32[sl]], axis=1).astype(BF16)
        cq = np.tile(c32[sl].T, (4, 1)).astype(BF16)                    # [128, NLOC]
        s2q = np.tile(np.concatenate([-s32[sl].T, s32[sl].T], 0), (2, 1)).astype(BF16)
        halves.append((ck, s2k, cq, s2q))

    # one-hot G-broadcast lhsT: eblk[c][h, p] = 1 iff head h owns partition p
    # of chunk c (heads 2c: p<64, 2c+1: p>=64)
    eblk = np.zeros((6, NUM_HEADS, 128), dtype=BF16)
    for cc in range(6):
        eblk[cc, 2 * cc, :64] = 1.0
        eblk[cc, 2 * cc + 1, 64:] = 1.0

    in_maps = []
    for core in range(NCORES):
        b = core // 2
        hi = core % 2
        sl = slice(hi * NLOC, (hi + 1) * NLOC)
        ck, s2k, cq, s2q = halves[hi]
        in_maps.append({
            "xq": np.ascontiguousarray(query[b, sl].T).astype(BF16),
            "xk": np.ascontiguousarray(key[b, sl].T).astype(BF16),
            "xv": np.ascontiguousarray(value[b, sl].T).astype(BF16),
            "wq": wq, "wk": wk, "wv": wv, "wo": wo,
            "bqp1": bqp1, "bkp1_b": bkp1_b, "bv_b": bv_b, "bo": bof,
            "cos_k": ck, "s2_k": s2k, "cos_q": cq, "s2_q": s2q,
            "eblk": eblk,
        })
    return in_maps


# --------------------------------------------------------------------------
# device kernel
# --------------------------------------------------------------------------

def _build_nc():
    import concourse.bacc as bacc
    import concourse.mybir as mybir
    import concourse.tile as tile

    fp32 = mybir.dt.float32
    bf16 = mybir.dt.bfloat16
    AF = mybir.ActivationFunctionType
    OP = mybir.AluOpType

    nc = bacc.Bacc("TRN2", target_bir_lowering=False, debug=False,
                   num_devices=NCORES)

    def din(name, shape, dt=bf16):
        return nc.dram_tensor(name, shape, dt, kind="ExternalInput").ap()

    xq = din("xq", [C, NLOC])
    xk = din("xk", [C, NLOC])
    xv = din("xv", [C, NLOC])
    wq_d = din("wq", [C, C])
    wk_d = din("wk", [C, C])
    wv_d = din("wv", [C, C])
    wo_d = din("wo", [C, C])
    bqp1_d = din("bqp1", [C])
    bkp1_b_d = din("bkp1_b", [128, C])
    bv_b_d = din("bv_b", [128, C])
    bo_d = din("bo", [C], fp32)
    cos_k_d = din("cos_k", [NLOC, 64])
    s2_k_d = din("s2_k", [NLOC, 64])
    cos_q_d = din("cos_q", [128, NLOC])
    s2_q_d = din("s2_q", [128, NLOC])
    eblk_d = din("eblk", [6, NUM_HEADS, 128])
    fp16 = mybir.dt.float16
    outT = nc.dram_tensor("outT", [C, NLOC], fp16, kind="ExternalOutput").ap()
    DEBUG = os.environ.get("KERNEL_DEBUG") == "1"
    if DEBUG:
        dbg_qrope = nc.dram_tensor("dbg_qrope", [128, 6, NLOC], bf16, kind="ExternalOutput").ap()
        dbg_res = nc.dram_tensor("dbg_res", [128, 6, NLOC], bf16, kind="ExternalOutput").ap()
        dbg_zb = nc.dram_tensor("dbg_zb", [NUM_HEADS, NLOC], bf16, kind="ExternalOutput").ap()
        dbg_sums = nc.dram_tensor("dbg_sums", [128, 12], fp32, kind="ExternalOutput").ap()
        dbg_kv2 = nc.dram_tensor("dbg_kv2", [128, 6, 128], bf16, kind="ExternalOutput").ap()
        dbg_wneg = nc.dram_tensor("dbg_wneg", [NUM_HEADS, C], bf16, kind="ExternalOutput").ap()
        dbg_kblock = nc.dram_tensor("dbg_kblock", [128, 6, NUM_HEADS], bf16, kind="ExternalOutput").ap()

    SFAC = SCALE / N       # kv and z scale
    NF = NLOC // 512       # 4 n-slices of 512
    NK = NLOC // 128       # 16 chunks of 128 for k/v phase
    NPAY = 65 * 768 + 128 * 12   # collective payload in fp32 elems

    with tile.TileContext(nc) as tc:
        from contextlib import ExitStack
        with ExitStack() as ctx:
            consts = ctx.enter_context(tc.tile_pool(name="consts", bufs=1))
            resid = ctx.enter_context(tc.tile_pool(name="resid", bufs=1))
            xin = ctx.enter_context(tc.tile_pool(name="xin", bufs=3))
            work = ctx.enter_context(tc.tile_pool(name="work", bufs=2))
            single = ctx.enter_context(tc.tile_pool(name="single", bufs=1))
            big = ctx.enter_context(tc.tile_pool(name="big", bufs=1))
            gsb = ctx.enter_context(tc.tile_pool(name="gsb", bufs=4))
            dram = ctx.enter_context(tc.tile_pool(name="dram", bufs=1, space="DRAM"))
            ph1 = ExitStack()
            pps = ph1.enter_context(tc.tile_pool(name="pps", bufs=4, space="PSUM"))
            kvp = ph1.enter_context(tc.tile_pool(name="kvp", bufs=1, space="PSUM"))

            # ---- phase-1-critical constants. xk0/xv0 ride the FRONT of the
            # scalar queue so their 196KB isn't stuck behind 2.4MB of weights
            # on the sync side; wk order (which empirically wins) unchanged ----
            xk0 = xin.tile([128, 6, 128], bf16, tag="xk_t", name="xk0")
            nc.scalar.dma_start(xk0[:], xk[:, 0:128].rearrange("(s p) n -> p s n", p=128))
            xv0 = xin.tile([128, 6, 128], bf16, tag="xv_t", name="xv0")
            nc.scalar.dma_start(xv0[:], xv[:, 0:128].rearrange("(s p) n -> p s n", p=128))
            wk_s = consts.tile([128, 6, C], bf16, tag="wk")
            for s in range(6):
                nc.sync.dma_start(wk_s[:, s, :],
                                  wk_d.rearrange("(s p) o -> p s o", p=128)[:, s, :])
            bkb_s = consts.tile([128, C], bf16, tag="bkb")
            nc.sync.dma_start(bkb_s[:], bkp1_b_d[:])
            bvb_s = consts.tile([128, C], bf16, tag="bvb")
            nc.scalar.dma_start(bvb_s[:], bv_b_d[:])
            cos_k_s = consts.tile([128, NK, 64], bf16, tag="cos_k")
            nc.scalar.dma_start(cos_k_s[:], cos_k_d.rearrange("(j p) d -> p j d", p=128))
            s2_k_s = consts.tile([128, NK, 64], bf16, tag="s2_k")
            nc.scalar.dma_start(s2_k_s[:], s2_k_d.rearrange("(j p) d -> p j d", p=128))
            wv_s = consts.tile([128, 6, C], bf16, tag="wv")
            for s in range(6):
                nc.scalar.dma_start(wv_s[:, s, :],
                                    wv_d.rearrange("(s p) o -> p s o", p=128)[:, s, :])
            xk1 = xin.tile([128, 6, 128], bf16, tag="xk_t", name="xk1")
            nc.sync.dma_start(xk1[:], xk[:, 128:256].rearrange("(s p) n -> p s n", p=128))
            xv1 = xin.tile([128, 6, 128], bf16, tag="xv_t", name="xv1")
            nc.sync.dma_start(xv1[:], xv[:, 128:256].rearrange("(s p) n -> p s n", p=128))
            bo_s = consts.tile([128, 6], fp32, tag="bo")
            nc.scalar.dma_start(bo_s[:], bo_d.rearrange("(s p) -> p s", p=128))

            ones_row = consts.tile([1, 512], bf16, tag="ones_row")
            nc.vector.memset(ones_row[:], 1.0)
            negone = consts.tile([128, 1], fp32, tag="negone")
            nc.vector.memset(negone[:], -1.0)
            zero_col = consts.tile([1, 128], bf16, tag="zero_col")
            nc.vector.memset(zero_col[:], 0.0)
            ones_f = consts.tile([1, 24], fp32, tag="ones_f")
            nc.vector.memset(ones_f[:], 1.0)

            # ---- persistent tiles ----
            qpre = big.tile([128, 6, NLOC], bf16, tag="qbig", name="qpre")
            qrope = resid.tile([128, 6, NLOC], bf16, tag="qrope")
            # res aliases qpre: qpre's last reader is the z matmul chain,
            # which completes (program order) before res[:, cc, nsl] writes.
            res = qpre
            zb = resid.tile([NUM_HEADS, NLOC], bf16, tag="zb")
            kv2f = resid.tile([128, 6, 128], fp32, tag="kv2f")
            nc.vector.memset(kv2f[:], 0.0)
            kv2sb = resid.tile([128, 6, 128], bf16, tag="kv2sb")
            kblock = resid.tile([128, 6, NUM_HEADS], bf16, tag="kblock")
            nc.vector.memset(kblock[:], 0.0)
            vbneg = resid.tile([128, 6, NUM_HEADS], bf16, tag="vbneg")
            nc.vector.memset(vbneg[:], 0.0)
            wneg = resid.tile([NUM_HEADS, C], bf16, tag="wneg")

            # kv psums: 3 banks, persist through phase 1.
            # head h accumulates at [0:65, (h%4)*128 : +128] of tile h//4.
            # start=True clears the whole bank's has_written bits, so packing 4
            # heads' accumulation groups per bank needs a single bank-wide
            # zero-write group opener; all kv matmuls then accumulate.
            kvps = [kvp.tile([128, 512], fp32, tag=f"kvps{t}", name=f"kvps{t}")
                    for t in range(3)]
            for t in range(3):
                nc.tensor.matmul(kvps[t][0:65, :], zero_col[:, 0:65], ones_row[:],
                                 start=True, stop=False, skip_group_check=True)
            sums_ps = kvp.tile([128, 12], fp32, tag="sums_ps")

            # ================= phase 1: k/v proj, elu, rope, kv =================
            for j in range(NK):
                if j == 0:
                    xk_t, xv_t = xk0, xv0
                elif j == 1:
                    xk_t, xv_t = xk1, xv1
                else:
                    xk_t = xin.tile([128, 6, 128], bf16, tag="xk_t",
                                    name=f"xk{j}")
                    nc.sync.dma_start(
                        xk_t[:], xk[:, j * 128:(j + 1) * 128]
                        .rearrange("(s p) n -> p s n", p=128))
                    xv_t = xin.tile([128, 6, 128], bf16, tag="xv_t",
                                    name=f"xv{j}")
                    nc.sync.dma_start(
                        xv_t[:], xv[:, j * 128:(j + 1) * 128]
                        .rearrange("(s p) n -> p s n", p=128))

                vk = work.tile([128, NUM_HEADS, 128], bf16, tag="vk")
                e_t = work.tile([128, C], bf16, tag="e_t")
                tk = work.tile([128, C], bf16, tag="tk")
                kra = work.tile([128, NUM_HEADS, 66], bf16, tag="kra")
                nc.vector.memset(kra[:, :, 64:65], 1.0)

                # k projection (no bias opener; bias via DVE add below)
                psk = [pps.tile([128, 384], fp32, tag="pp384", name=f"psk{half}")
                       for half in range(2)]
                for s in range(6):
                    for half in range(2):
                        nc.tensor.matmul(psk[half][:], xk_t[:, s, :],
                                         wk_s[:, s, half * 384:(half + 1) * 384],
                                         start=(s == 0), stop=(s == 5))
                for half in range(2):
                    osl = slice(half * 384, (half + 1) * 384)
                    hsl = slice(half * 6, (half + 1) * 6)
                    # tk = x + 1 (bias tile holds bk+1)
                    nc.vector.tensor_tensor(tk[:, osl], psk[half][:],
                                            bkb_s[:, osl], OP.add)
                    nc.scalar.activation(e_t[:, osl], tk[:, osl], AF.Exp,
                                         bias=negone[:])
                    nc.vector.scalar_tensor_tensor(
                        vk[:, hsl, 64:128],
                        e_t[:, osl].rearrange("p (h e) -> p h e", e=64),
                        1.0, tk[:, osl].rearrange("p (h e) -> p h e", e=64),
                        OP.min, OP.max)

                # v projection (bias via DVE add)
                psv = [pps.tile([128, 384], fp32, tag="pp384", name=f"psv{half}")
                       for half in range(2)]
                for s in range(6):
                    for half in range(2):
                        nc.tensor.matmul(psv[half][:], xv_t[:, s, :],
                                         wv_s[:, s, half * 384:(half + 1) * 384],
                                         start=(s == 0), stop=(s == 5))
                for half in range(2):
                    osl = slice(half * 384, (half + 1) * 384)
                    hsl = slice(half * 6, (half + 1) * 6)
                    nc.vector.tensor_tensor(
                        vk[:, hsl, 0:64],
                        psv[half][:].rearrange("p (h e) -> p h e", e=64),
                        bvb_s[:, osl].rearrange("p (h e) -> p h e", e=64),
                        OP.add)

                # rope on k_pre -> kra[:, :, 0:64]; s2 mults on gpsimd
                kpre_v = vk[:, :, 64:128]
                cosj = cos_k_s[:, j, None, :].to_broadcast([128, NUM_HEADS, 64])
                nc.vector.tensor_tensor(kra[:, :, 0:64], kpre_v, cosj, OP.mult)
                tmpb = work.tile([128, NUM_HEADS, 64], bf16, tag="tmpb")
                s2t = s2_k_s[:, j, None, 0:32].to_broadcast([128, NUM_HEADS, 32])
                s2b = s2_k_s[:, j, None, 32:64].to_broadcast([128, NUM_HEADS, 32])
                nc.gpsimd.tensor_tensor(tmpb[:, :, 0:32], vk[:, :, 96:128], s2t, OP.mult)
                nc.gpsimd.tensor_tensor(tmpb[:, :, 32:64], vk[:, :, 64:96], s2b, OP.mult)
                nc.vector.tensor_tensor(kra[:, :, 0:64], kra[:, :, 0:64], tmpb[:],
                                        OP.add)

                # kv accumulation: [k_rope | 1]^T @ [v | k_pre] per head
                for h in range(NUM_HEADS):
                    nc.tensor.matmul(
                        kvps[h // 4][0:65, (h % 4) * 128:(h % 4) * 128 + 128],
                        kra[:, h, 0:65], vk[:, h, :],
                        start=False, stop=(j == NK - 1), skip_group_check=True)

            # ====== phase 1.5: slim payload + pre-transposed ksums, collective
            # v-halves of kv psums (rows 0:65; row 64 = vsum) -> [65, 3, 256]
            kvsb_v = single.tile([65, 3, 256], fp32, tag="kvsb_v")
            for t in range(3):
                nc.scalar.activation(
                    kvsb_v[:, t, :].rearrange("p (m e) -> p m e", e=64),
                    kvps[t][0:65, :].rearrange("p (m c) -> p m c", c=128)[:, :, 0:64],
                    AF.Copy)
            # full sums row (row 64: [vsum_h | ksum_h] per 128-col head block)
            kvs_row = single.tile([1, 1536], fp32, tag="kvs_row")
            for t in range(3):
                nc.scalar.activation(kvs_row[:, t * 512:(t + 1) * 512],
                                     kvps[t][64:65, :], AF.Copy)
            # transpose ksums to columns via K=1 matmuls; col h = ksum_h,
            # valid at partitions (h%2)*64 .. +64 (vsums transposed post-CC)
            for h in range(NUM_HEADS):
                wk_off = 128 * h + 64 - 64 * (h % 2)
                nc.tensor.matmul(sums_ps[:, h:h + 1],
                                 kvs_row[:, wk_off:wk_off + 128],
                                 ones_f[:, 0:1],
                                 start=True, stop=True, skip_group_check=True)
            sums_sb = single.tile([128, 12], fp32, tag="sums_sb")
            nc.scalar.activation(sums_sb[:], sums_ps[:], AF.Copy)

            bounce_in = dram.tile([NPAY], fp32, tag="b_in")
            bounce_out = dram.tile([NPAY], fp32, tag="b_out")
            nc.sync.dma_start(
                bounce_in[0:65 * 768].rearrange("(p f) -> p f", p=65),
                kvsb_v.rearrange("p a b -> p (a b)"))
            nc.sync.dma_start(
                bounce_in[65 * 768:NPAY].rearrange("(p f) -> p f", p=128),
                sums_sb[:])
            nc.gpsimd.collective_compute(
                "AllReduce", OP.add,
                replica_groups=[[0, 1], [2, 3], [4, 5], [6, 7]],
                ins=[bounce_in.opt()], outs=[bounce_out.opt()])

            # ---- late consts (needed from phase 2a on) ----
            wq_s = consts.tile([128, 6, C], bf16, tag="wq")
            nc.sync.dma_start(wq_s[:], wq_d.rearrange("(s p) o -> p s o", p=128))
            wo_s = consts.tile([128, 6, C], bf16, tag="wo")
            nc.sync.dma_start(wo_s[:], wo_d.rearrange("(s p) o -> p s o", p=128))
            bqp1_s = consts.tile([1, C], bf16, tag="bqp1")
            nc.sync.dma_start(bqp1_s[:], bqp1_d[None, :])
            cos_q_s = consts.tile([128, NLOC], bf16, tag="cos_q")
            nc.sync.dma_start(cos_q_s[:], cos_q_d[:])
            s2_q_s = consts.tile([128, NLOC], bf16, tag="s2_q")
            nc.sync.dma_start(s2_q_s[:], s2_q_d[:])
            eblk_s = consts.tile([NUM_HEADS, 6, 128], bf16, tag="eblk")
            nc.sync.dma_start(eblk_s[:], eblk_d.rearrange("c h p -> h c p"))

            # ================= phase 2a: q proj, elu, rope =================
            for nq in range(NF):
                nsl = slice(nq * 512, (nq + 1) * 512)
                xq_t = xin.tile([128, 6, 512], bf16, tag="xq_t")
                nc.sync.dma_start(
                    xq_t[:], xq[:, nsl].rearrange("(s p) n -> p s n", p=128))
                for oc in range(6):
                    psq = pps.tile([128, 512], fp32, tag="pp384", name="psq")
                    nc.tensor.matmul(psq[:], bqp1_s[:, oc * 128:(oc + 1) * 128],
                                     ones_row[:], start=True, stop=False)
                    for s in range(6):
                        nc.tensor.matmul(psq[:], wq_s[:, s, oc * 128:(oc + 1) * 128],
                                         xq_t[:, s, :], start=False, stop=(s == 5))
                    e_q = work.tile([128, 512], bf16, tag="e_q")
                    nc.scalar.activation(e_q[:], psq[:], AF.Exp, bias=negone[:])
                    nc.vector.scalar_tensor_tensor(
                        qpre[:, oc, nsl], e_q[:], 1.0, psq[:], OP.min, OP.max)

                # rope: A + B with B reading the 32-block-swapped q_pre
                qsw = work.tile([128, 6, 512], bf16, tag="qsw")
                for g4 in range(4):
                    sp = (g4 ^ 1) * 32
                    nc.sync.dma_start(qsw[g4 * 32:(g4 + 1) * 32, :, :],
                                      qpre[sp:sp + 32, :, nsl])
                for oc in range(6):
                    nc.vector.tensor_tensor(qrope[:, oc, nsl], qpre[:, oc, nsl],
                                            cos_q_s[:, nsl], OP.mult)
                    tmpq = work.tile([128, 512], bf16, tag="tmpq")
                    nc.vector.tensor_tensor(tmpq[:], qsw[:, oc, :], s2_q_s[:, nsl],
                                            OP.mult)
                    nc.vector.tensor_tensor(qrope[:, oc, nsl], qrope[:, oc, nsl],
                                            tmpq[:], OP.add)

            # ================= phase 2b: post-collective assembly =============
            # post-CC loads ride the scalar queue: the Act engine is idle
            # after its 2a Exps and can park at the CC-wait, while the sync
            # queue is still grinding slice-3 swap/input DMAs
            kvall_v = single.tile([65, 768], fp32, tag="kvall_v")
            nc.scalar.dma_start(kvall_v[:],
                                bounce_out[0:65 * 768].rearrange("(p f) -> p f", p=65))
            sums_all = single.tile([128, 12], fp32, tag="sums_all")
            nc.scalar.dma_start(sums_all[:],
                                bounce_out[65 * 768:NPAY].rearrange("(p f) -> p f", p=128))

            # kblock (z weights, * SFAC) via Act so it bypasses the DVE rope
            # backlog: col h=2s+t of chunk s <- ksum col h
            kb_flat = kblock.rearrange("p s h -> p (s h)")
            vb_flat = vbneg.rearrange("p s h -> p (s h)")
            for t in range(2):
                psl = slice(t * 64, (t + 1) * 64)
                nc.scalar.activation(
                    kb_flat[psl, t:t + 71:14], sums_all[psl, t:12:2],
                    AF.Copy, scale=SFAC)

            # kv2sb: block-diag [kv_2c, kv_2c+1] per chunk, * SFAC
            kva_r = kvall_v[0:64, :].rearrange("p (cc two e) -> p cc two e",
                                               two=2, e=64)
            for t in range(2):
                nc.scalar.dma_start(
                    kv2f[t * 64:(t + 1) * 64, :, t * 64:t * 64 + 64],
                    kva_r[:, :, t, :])
            nc.scalar.activation(kv2sb[:], kv2f[:], AF.Copy, scale=SFAC)

            ph1.close()
            zpool = ctx.enter_context(tc.tile_pool(name="zpool", bufs=2, space="PSUM"))
            gpool = ctx.enter_context(tc.tile_pool(name="gpool", bufs=2, space="PSUM"))
            apool = ctx.enter_context(tc.tile_pool(name="apool", bufs=2, space="PSUM"))
            opool = ctx.enter_context(tc.tile_pool(name="opool", bufs=2, space="PSUM"))

            # ========== phase 2c: z chains for all slices first ==========
            gbs = []
            for nq in range(NF):
                nsl = slice(nq * 512, (nq + 1) * 512)
                psz = zpool.tile([128, 512], fp32, tag="zp", name=f"psz{nq}")[0:NUM_HEADS, :]
                for s in range(6):
                    nc.tensor.matmul(psz[:], kblock[:, s, :], qpre[:, s, nsl],
                                     start=(s == 0), stop=(s == 5))
                gf = work.tile([NUM_HEADS, 512], fp32, tag="gf")
                nc.vector.reciprocal_approx_fast(gf[:], psz[:])  # z >= 7, eps moot
                gb = gsb.tile([NUM_HEADS, 512], bf16, tag="gb", name=f"gb{nq}")
                nc.vector.tensor_scalar_add(gb[:], gf[:], 1.0)
                nc.scalar.activation(zb[:, nsl], psz[:], AF.Copy)
                gbs.append(gb)

            # vsum columns (post-CC transposes off the trigger path), then
            # vbneg assembly and wneg = -(W~)^T : [12, 768]
            kva_row = single.tile([1, 768], fp32, tag="kva_row")
            nc.scalar.activation(kva_row[:], kvall_v[64:65, :], AF.Copy)
            vsum_ps = zpool.tile([128, 512], fp32, tag="zp", name="vsum_ps")[:, 0:12]
            for h in range(NUM_HEADS):
                wv_off = 64 * h - 64 * (h % 2)
                nc.tensor.matmul(vsum_ps[:, h:h + 1],
                                 kva_row[:, wv_off:wv_off + 128],
                                 ones_f[:, 0:1],
                                 start=True, stop=True, skip_group_check=True)
            for t in range(2):
                psl = slice(t * 64, (t + 1) * 64)
                nc.scalar.activation(
                    vb_flat[psl, t:t + 71:14], vsum_ps[psl, t:12:2],
                    AF.Copy, scale=-1.0 / N)
            for half in range(2):
                osl = slice(half * 384, (half + 1) * 384)
                psw = zpool.tile([128, 512], fp32, tag="zp", name=f"psw{half}")[0:NUM_HEADS, 0:384]
                for s in range(6):
                    nc.tensor.matmul(psw[:], vbneg[:, s, :], wo_s[:, s, osl],
                                     start=(s == 0), stop=(s == 5))
                nc.scalar.activation(wneg[:, osl], psw[:], AF.Copy)

            # ====== phase 3+4: per-slice G, q~, qkv, o_proj (pipelined) ======
            for nq in range(NF):
                nsl = slice(nq * 512, (nq + 1) * 512)
                for cc in range(6):
                    psg = gpool.tile([128, 512], fp32, tag="gp", name=f"psg{nq}_{cc}")
                    nc.tensor.matmul(psg[:], eblk_s[:, cc, :], gbs[nq][:],
                                     start=True, stop=True)
                    nc.vector.tensor_tensor(qrope[:, cc, nsl], qrope[:, cc, nsl],
                                            psg[:], OP.mult)
                    psa = apool.tile([128, 512], fp32, tag="ap", name=f"psa{nq}_{cc}")
                    nc.tensor.matmul(psa[:], kv2sb[:, cc, :], qrope[:, cc, nsl],
                                     start=True, stop=True)
                    nc.scalar.activation(res[:, cc, nsl], psa[:], AF.Copy)
                # o_proj for this n-slice
                for c2 in range(6):
                    c2sl = slice(c2 * 128, (c2 + 1) * 128)
                    pso = opool.tile([128, 512], fp32, tag="op", name=f"pso{nq}_{c2}")
                    nc.tensor.matmul(pso[:], wneg[:, c2sl], zb[:, nsl],
                                     start=True, stop=False)
                    for s in range(6):
                        nc.tensor.matmul(pso[:], wo_s[:, s, c2sl], res[:, s, nsl],
                                         start=False, stop=(s == 5))
                    osb = work.tile([128, 512], fp16, tag="osb")
                    nc.scalar.activation(osb[:], pso[:], AF.Identity,
                                         bias=bo_s[:, c2:c2 + 1])
                    nc.sync.dma_start(outT[c2sl, nsl], osb[:])

            if DEBUG:
                nc.sync.dma_start(dbg_res[:], res[:])
                nc.sync.dma_start(dbg_qrope[:], qrope[:])
                nc.sync.dma_start(dbg_zb[:], zb[:])
                nc.sync.dma_start(dbg_sums[:], sums_all[:])
                nc.sync.dma_start(dbg_kv2[:], kv2sb[:])
                nc.sync.dma_start(dbg_wneg[:], wneg[:])
                nc.sync.dma_start(dbg_kblock[:], kblock[:])

    nc.compile()
    return nc


def _get_nc():
    if "nc" not in _CACHE:
        _CACHE["nc"] = _build_nc()
    return _CACHE["nc"]


# --------------------------------------------------------------------------
# entry point
# --------------------------------------------------------------------------

def kernel(query, key, value, Wq, bq, Wk, bk, Wv, bv, Wo, bo, H, W):
    from concourse.bass_utils import run_bass_kernel_spmd

    assert int(H) == 64 and int(W) == 64
    query = np.asarray(query, np.float32)
    key = np.asarray(key, np.float32)
    value = np.asarray(value, np.float32)
    in_maps = _build_host_inputs(
        query, key, value,
        np.asarray(Wq, np.float32), np.asarray(bq, np.float32),
        np.asarray(Wk, np.float32), np.asarray(bk, np.float32),
        np.asarray(Wv, np.float32), np.asarray(bv, np.float32),
        np.asarray(Wo, np.float32), np.asarray(bo, np.float32))

    nc = _get_nc()
    kwargs = {}
    if os.environ.get("KERNEL_TRACE") == "1":
        kwargs = dict(trace=True, tmpdir=tempfile.mkdtemp(prefix="malat_"))
    r = run_bass_kernel_spmd(nc, in_maps, core_ids=list(range(NCORES)), **kwargs)
    LAST_RESULTS[0] = r

    out = np.empty((B, N, C), np.float32)
    for core in range(NCORES):
        b = core // 2
        sl = slice((core % 2) * NLOC, (core % 2 + 1) * NLOC)
        out[b, sl, :] = r.results[core]["outT"].T.astype(np.float32)
    return out


# revision 26
# speedup vs baseline: 1.0313x; 1.0313x over previous
"""MALA attention (linear attention w/ 2D RoPE + magnitude term) on 8 trn2 cores.

Sharding: core i handles batch b = i//2, sequence rows (i%2)*2048..+2048.
Cross-core data (kv = k_rope^T v, k_sum, v_sum -- all sums over the full
sequence) is combined with a pairwise AllReduce (~210KB). Everything else is
local. bf16 operands everywhere with fp32 PSUM accumulation.

Math (per batch b, head h, reference semantics):
  q = elu(query @ Wq.T + bq) + 1      (same k with Wk/bk; v plain)
  z = (q . mean_n(k)) * d^-0.5
  q, k <- rope(q), rope(k)
  kv = k^T v * (d^-0.5 / N)
  res = (q @ kv) * (1 + 1/(z+1e-6)) - z * mean_n(v)
  out = res @ Wo.T + bo

Device-side formulation:
  elu(x)+1 = max(x + 1, min(exp(x), 1))    [exact: exp(min(x,0)) = min(exp x,1)]
  k/v bias (+1 for k) added via DVE from pre-broadcast [128,C] bias tiles;
  q bias via per-partition Act bias (features on partitions there). All
  bias-opener matmuls eliminated.
  g = 1 + 1/(z+eps) folded into q_rope before q@kv (G built by one-hot matmul).
  -z*v_mean term folded into o_proj as an extra matmul (-W~)^T @ z where
  W~[c,h] = Wo[:, head h] @ v_mean_h.
  The within-head d-index is permuted (evens first) on Wq/Wk columns + trig
  tables so rotate_every_two becomes a 32-column block swap.
  Collective payload is slimmed to kv v-halves+vsum row [65,768] + per-head
  ksums pre-transposed to columns [128,12] (12 tiny K=1 matmuls; vsum
  transposes happen post-collective, off the trigger path), so the
  post-collective critical path is 2 DMAs + 2 strided Act copies.
"""

import os
import tempfile

import numpy as np
import ml_dtypes

NUM_HEADS = 12
B, N, C = 4, 4096, 768
D = 64
NCORES = 8
NLOC = N // 2          # rows per core
SCALE = D ** -0.5
BF16 = ml_dtypes.bfloat16

_CACHE = {}
LAST_RESULTS = [None]  # test.py reads profiling info from here


# --------------------------------------------------------------------------
# host-side helpers
# --------------------------------------------------------------------------

def _perm64():
    # evens first, then odds (within each head's 64 dims)
    return np.concatenate([np.arange(0, 64, 2), np.arange(1, 64, 2)])


def _trig_tables():
    """c32/s32: [N, 32] fp32, value of cos/sin at original dim 2i (== 2i+1)."""
    H = W = 64
    angle = 1.0 / (10000.0 ** np.linspace(0.0, 1.0, D // 4))
    angle = np.repeat(angle, 2)                          # [32]
    ih = np.arange(H, dtype=np.float64)[:, None] * angle[None, :]   # [H, 32]
    iw = np.arange(W, dtype=np.float64)[:, None] * angle[None, :]
    sin_h, cos_h = np.sin(ih), np.cos(ih)
    sin_w, cos_w = np.sin(iw), np.cos(iw)
    r = np.arange(N) // W
    c = np.arange(N) % W
    s_full = np.concatenate([sin_h[r], sin_w[c]], axis=1)   # [N, 64]
    c_full = np.concatenate([cos_h[r], cos_w[c]], axis=1)
    c32 = c_full[:, 0::2].astype(np.float32)
    s32 = s_full[:, 0::2].astype(np.float32)
    return c32, s32


def _build_host_inputs(query, key, value, Wq, bq, Wk, bk, Wv, bv, Wo, bo):
    p64 = _perm64()
    perm = (np.arange(NUM_HEADS)[:, None] * 64 + p64[None, :]).reshape(-1)

    wq = np.ascontiguousarray(Wq.T[:, perm]).astype(BF16)
    wk = np.ascontiguousarray(Wk.T[:, perm]).astype(BF16)
    wv = np.ascontiguousarray(Wv.T).astype(BF16)
    wo = np.ascontiguousarray(Wo.T).astype(BF16)
    bqp1 = (bq[perm] + 1.0).astype(BF16)
    # k/v biases pre-broadcast across partitions for the DVE bias add
    bkp1_b = np.tile((bk[perm] + 1.0).astype(BF16)[None, :], (128, 1))
    bv_b = np.tile(bv.astype(BF16)[None, :], (128, 1))
    bof = bo.astype(np.float32)

    c32, s32 = _trig_tables()
    halves = []
    for hi in range(2):
        sl = slice(hi * NLOC, (hi + 1) * NLOC)
        ck = np.concatenate([c32[sl], c32[sl]], axis=1).astype(BF16)    # [NLOC, 64]
        s2k = np.concatenate([-s32[sl], s32[sl]], axis=1).astype(BF16)
        cq = np.tile(c32[sl].T, (4, 1)).astype(BF16)                    # [128, NLOC]
        s2q = np.tile(np.concatenate([-s32[sl].T, s32[sl].T], 0), (2, 1)).astype(BF16)
        halves.append((ck, s2k, cq, s2q))

    # one-hot G-broadcast lhsT: eblk[c][h, p] = 1 iff head h owns partition p
    # of chunk c (heads 2c: p<64, 2c+1: p>=64)
    eblk = np.zeros((6, NUM_HEADS, 128), dtype=BF16)
    for cc in range(6):
        eblk[cc, 2 * cc, :64] = 1.0
        eblk[cc, 2 * cc + 1, 64:] = 1.0

    in_maps = []
    for core in range(NCORES):
        b = core // 2
        hi = core % 2
        sl = slice(hi * NLOC, (hi + 1) * NLOC)
        ck, s2k, cq, s2q = halves[hi]
        in_maps.append({
            "xq": np.ascontiguousarray(query[b, sl].T).astype(BF16),
            "xk": np.ascontiguousarray(key[b, sl].T).astype(BF16),
            "xv": np.ascontiguousarray(value[b, sl].T).astype(BF16),
            "wq": wq, "wk": wk, "wv": wv, "wo": wo,
            "bqp1": bqp1, "bkp1_b": bkp1_b, "bv_b": bv_b, "bo": bof,
            "cos_k": ck, "s2_k": s2k, "cos_q": cq, "s2_q": s2q,
            "eblk": eblk,
        })
    return in_maps


# --------------------------------------------------------------------------
# device kernel
# --------------------------------------------------------------------------

def _build_nc():
    import concourse.bacc as bacc
    import concourse.mybir as mybir
    import concourse.tile as tile

    fp32 = mybir.dt.float32
    bf16 = mybir.dt.bfloat16
    AF = mybir.ActivationFunctionType
    OP = mybir.AluOpType

    nc = bacc.Bacc("TRN2", target_bir_lowering=False, debug=False,
                   num_devices=NCORES)

    def din(name, shape, dt=bf16):
        return nc.dram_tensor(name, shape, dt, kind="ExternalInput").ap()

    xq = din("xq", [C, NLOC])
    xk = din("xk", [C, NLOC])
    xv = din("xv", [C, NLOC])
    wq_d = din("wq", [C, C])
    wk_d = din("wk", [C, C])
    wv_d = din("wv", [C, C])
    wo_d = din("wo", [C, C])
    bqp1_d = din("bqp1", [C])
    bkp1_b_d = din("bkp1_b", [128, C])
    bv_b_d = din("bv_b", [128, C])
    bo_d = din("bo", [C], fp32)
    cos_k_d = din("cos_k", [NLOC, 64])
    s2_k_d = din("s2_k", [NLOC, 64])
    cos_q_d = din("cos_q", [128, NLOC])
    s2_q_d = din("s2_q", [128, NLOC])
    eblk_d = din("eblk", [6, NUM_HEADS, 128])
    fp16 = mybir.dt.float16
    outT = nc.dram_tensor("outT", [C, NLOC], fp16, kind="ExternalOutput").ap()
    DEBUG = os.environ.get("KERNEL_DEBUG") == "1"
    if DEBUG:
        dbg_qrope = nc.dram_tensor("dbg_qrope", [128, 6, NLOC], bf16, kind="ExternalOutput").ap()
        dbg_res = nc.dram_tensor("dbg_res", [128, 6, NLOC], bf16, kind="ExternalOutput").ap()
        dbg_zb = nc.dram_tensor("dbg_zb", [NUM_HEADS, NLOC], bf16, kind="ExternalOutput").ap()
        dbg_sums = nc.dram_tensor("dbg_sums", [128, 12], fp32, kind="ExternalOutput").ap()
        dbg_kv2 = nc.dram_tensor("dbg_kv2", [128, 6, 128], bf16, kind="ExternalOutput").ap()
        dbg_wneg = nc.dram_tensor("dbg_wneg", [NUM_HEADS, C], bf16, kind="ExternalOutput").ap()
        dbg_kblock = nc.dram_tensor("dbg_kblock", [128, 6, NUM_HEADS], bf16, kind="ExternalOutput").ap()

    SFAC = SCALE / N       # kv and z scale
    NF = NLOC // 512       # 4 n-slices of 512
    NK = NLOC // 128       # 16 chunks of 128 for k/v phase
    NPAY = 65 * 768 + 128 * 12   # collective payload in fp32 elems

    with tile.TileContext(nc) as tc:
        from contextlib import ExitStack
        with ExitStack() as ctx:
            consts = ctx.enter_context(tc.tile_pool(name="consts", bufs=1))
            resid = ctx.enter_context(tc.tile_pool(name="resid", bufs=1))
            xin = ctx.enter_context(tc.tile_pool(name="xin", bufs=3))
            work = ctx.enter_context(tc.tile_pool(name="work", bufs=2))
            single = ctx.enter_context(tc.tile_pool(name="single", bufs=1))
            big = ctx.enter_context(tc.tile_pool(name="big", bufs=1))
            gsb = ctx.enter_context(tc.tile_pool(name="gsb", bufs=4))
            dram = ctx.enter_context(tc.tile_pool(name="dram", bufs=1, space="DRAM"))
            ph1 = ExitStack()
            pps = ph1.enter_context(tc.tile_pool(name="pps", bufs=4, space="PSUM"))
            kvp = ph1.enter_context(tc.tile_pool(name="kvp", bufs=1, space="PSUM"))

            # ---- phase-1-critical constants. xk0/xv0 ride the FRONT of the
            # scalar queue so their 196KB isn't stuck behind 2.4MB of weights
            # on the sync side; wk order (which empirically wins) unchanged ----
            xk0 = xin.tile([128, 6, 128], bf16, tag="xk_t", name="xk0")
            nc.scalar.dma_start(xk0[:], xk[:, 0:128].rearrange("(s p) n -> p s n", p=128))
            xv0 = xin.tile([128, 6, 128], bf16, tag="xv_t", name="xv0")
            nc.scalar.dma_start(xv0[:], xv[:, 0:128].rearrange("(s p) n -> p s n", p=128))
            wk_s = consts.tile([128, 6, C], bf16, tag="wk")
            for s in range(6):
                nc.sync.dma_start(wk_s[:, s, :],
                                  wk_d.rearrange("(s p) o -> p s o", p=128)[:, s, :])
            # wv rides sync right behind wk: the scalar queue's issue rate
            # (xk0/xv0/bvb/cos/s2 ahead) was gating v-proj(0) until ~23us
            wv_s = consts.tile([128, 6, C], bf16, tag="wv")
            for s in range(6):
                nc.sync.dma_start(wv_s[:, s, :],
                                  wv_d.rearrange("(s p) o -> p s o", p=128)[:, s, :])
            bkb_s = consts.tile([128, C], bf16, tag="bkb")
            nc.sync.dma_start(bkb_s[:], bkp1_b_d[:])
            bvb_s = consts.tile([128, C], bf16, tag="bvb")
            nc.scalar.dma_start(bvb_s[:], bv_b_d[:])
            cos_k_s = consts.tile([128, NK, 64], bf16, tag="cos_k")
            nc.scalar.dma_start(cos_k_s[:], cos_k_d.rearrange("(j p) d -> p j d", p=128))
            s2_k_s = consts.tile([128, NK, 64], bf16, tag="s2_k")
            nc.scalar.dma_start(s2_k_s[:], s2_k_d.rearrange("(j p) d -> p j d", p=128))
            xk1 = xin.tile([128, 6, 128], bf16, tag="xk_t", name="xk1")
            nc.sync.dma_start(xk1[:], xk[:, 128:256].rearrange("(s p) n -> p s n", p=128))
            xv1 = xin.tile([128, 6, 128], bf16, tag="xv_t", name="xv1")
            nc.sync.dma_start(xv1[:], xv[:, 128:256].rearrange("(s p) n -> p s n", p=128))
            bo_s = consts.tile([128, 6], fp32, tag="bo")
            nc.scalar.dma_start(bo_s[:], bo_d.rearrange("(s p) -> p s", p=128))

            ones_row = consts.tile([1, 512], bf16, tag="ones_row")
            nc.vector.memset(ones_row[:], 1.0)
            negone = consts.tile([128, 1], fp32, tag="negone")
            nc.vector.memset(negone[:], -1.0)
            zero_col = consts.tile([1, 128], bf16, tag="zero_col")
            nc.vector.memset(zero_col[:], 0.0)
            ones_f = consts.tile([1, 24], fp32, tag="ones_f")
            nc.vector.memset(ones_f[:], 1.0)

            # ---- persistent tiles ----
            qpre = big.tile([128, 6, NLOC], bf16, tag="qbig", name="qpre")
            qrope = resid.tile([128, 6, NLOC], bf16, tag="qrope")
            # res aliases qpre: qpre's last reader is the z matmul chain,
            # which completes (program order) before res[:, cc, nsl] writes.
            res = qpre
            zb = resid.tile([NUM_HEADS, NLOC], bf16, tag="zb")
            kv2f = resid.tile([128, 6, 128], fp32, tag="kv2f")
            nc.vector.memset(kv2f[:], 0.0)
            kv2sb = resid.tile([128, 6, 128], bf16, tag="kv2sb")
            kblock = resid.tile([128, 6, NUM_HEADS], bf16, tag="kblock")
            nc.vector.memset(kblock[:], 0.0)
            vbneg = resid.tile([128, 6, NUM_HEADS], bf16, tag="vbneg")
            nc.vector.memset(vbneg[:], 0.0)
            wneg = resid.tile([NUM_HEADS, C], bf16, tag="wneg")

            # kv psums: 3 banks, persist through phase 1.
            # head h accumulates at [0:65, (h%4)*128 : +128] of tile h//4.
            # start=True clears the whole bank's has_written bits, so packing 4
            # heads' accumulation groups per bank needs a single bank-wide
            # zero-write group opener; all kv matmuls then accumulate.
            kvps = [kvp.tile([128, 512], fp32, tag=f"kvps{t}", name=f"kvps{t}")
                    for t in range(3)]
            for t in range(3):
                nc.tensor.matmul(kvps[t][0:65, :], zero_col[:, 0:65], ones_row[:],
                                 start=True, stop=False, skip_group_check=True)
            sums_ps = kvp.tile([128, 12], fp32, tag="sums_ps")

            # ================= phase 1: k/v proj, elu, rope, kv =================
            for j in range(NK):
                if j == 0:
                    xk_t, xv_t = xk0, xv0
                elif j == 1:
                    xk_t, xv_t = xk1, xv1
                else:
                    xk_t = xin.tile([128, 6, 128], bf16, tag="xk_t",
                                    name=f"xk{j}")
                    nc.sync.dma_start(
                        xk_t[:], xk[:, j * 128:(j + 1) * 128]
                        .rearrange("(s p) n -> p s n", p=128))
                    xv_t = xin.tile([128, 6, 128], bf16, tag="xv_t",
                                    name=f"xv{j}")
                    nc.sync.dma_start(
                        xv_t[:], xv[:, j * 128:(j + 1) * 128]
                        .rearrange("(s p) n -> p s n", p=128))

                vk = work.tile([128, NUM_HEADS, 128], bf16, tag="vk")
                e_t = work.tile([128, C], bf16, tag="e_t")
                tk = work.tile([128, C], bf16, tag="tk")
                kra = work.tile([128, NUM_HEADS, 66], bf16, tag="kra")
                nc.vector.memset(kra[:, :, 64:65], 1.0)

                # k projection (no bias opener; bias via DVE add below)
                psk = [pps.tile([128, 384], fp32, tag="pp384", name=f"psk{half}")
                       for half in range(2)]
                for s in range(6):
                    for half in range(2):
                        nc.tensor.matmul(psk[half][:], xk_t[:, s, :],
                                         wk_s[:, s, half * 384:(half + 1) * 384],
                                         start=(s == 0), stop=(s == 5))
                for half in range(2):
                    osl = slice(half * 384, (half + 1) * 384)
                    hsl = slice(half * 6, (half + 1) * 6)
                    # tk = x + 1 (bias tile holds bk+1)
                    nc.vector.tensor_tensor(tk[:, osl], psk[half][:],
                                            bkb_s[:, osl], OP.add)
                    nc.scalar.activation(e_t[:, osl], tk[:, osl], AF.Exp,
                                         bias=negone[:])
                    nc.vector.scalar_tensor_tensor(
                        vk[:, hsl, 64:128],
                        e_t[:, osl].rearrange("p (h e) -> p h e", e=64),
                        1.0, tk[:, osl].rearrange("p (h e) -> p h e", e=64),
                        OP.min, OP.max)

                # v projection (bias via DVE add)
                psv = [pps.tile([128, 384], fp32, tag="pp384", name=f"psv{half}")
                       for half in range(2)]
                for s in range(6):
                    for half in range(2):
                        nc.tensor.matmul(psv[half][:], xv_t[:, s, :],
                                         wv_s[:, s, half * 384:(half + 1) * 384],
                                         start=(s == 0), stop=(s == 5))
                for half in range(2):
                    osl = slice(half * 384, (half + 1) * 384)
                    hsl = slice(half * 6, (half + 1) * 6)
                    nc.vector.tensor_tensor(
                        vk[:, hsl, 0:64],
                        psv[half][:].rearrange("p (h e) -> p h e", e=64),
                        bvb_s[:, osl].rearrange("p (h e) -> p h e", e=64),
                        OP.add)

                # rope on k_pre -> kra[:, :, 0:64]; s2 mults on gpsimd
                kpre_v = vk[:, :, 64:128]
                cosj = cos_k_s[:, j, None, :].to_broadcast([128, NUM_HEADS, 64])
                nc.vector.tensor_tensor(kra[:, :, 0:64], kpre_v, cosj, OP.mult)
                tmpb = work.tile([128, NUM_HEADS, 64], bf16, tag="tmpb")
                s2t = s2_k_s[:, j, None, 0:32].to_broadcast([128, NUM_HEADS, 32])
                s2b = s2_k_s[:, j, None, 32:64].to_broadcast([128, NUM_HEADS, 32])
                nc.gpsimd.tensor_tensor(tmpb[:, :, 0:32], vk[:, :, 96:128], s2t, OP.mult)
                nc.gpsimd.tensor_tensor(tmpb[:, :, 32:64], vk[:, :, 64:96], s2b, OP.mult)
                nc.vector.tensor_tensor(kra[:, :, 0:64], kra[:, :, 0:64], tmpb[:],
                                        OP.add)

                # kv accumulation: [k_rope | 1]^T @ [v | k_pre] per head
                for h in range(NUM_HEADS):
                    nc.tensor.matmul(
                        kvps[h // 4][0:65, (h % 4) * 128:(h % 4) * 128 + 128],
                        kra[:, h, 0:65], vk[:, h, :],
                        start=False, stop=(j == NK - 1), skip_group_check=True)

            # ====== phase 1.5: slim payload + pre-transposed ksums, collective
            # v-halves of kv psums (rows 0:65; row 64 = vsum) -> [65, 3, 256]
            kvsb_v = single.tile([65, 3, 256], fp32, tag="kvsb_v")
            for t in range(3):
                nc.scalar.activation(
                    kvsb_v[:, t, :].rearrange("p (m e) -> p m e", e=64),
                    kvps[t][0:65, :].rearrange("p (m c) -> p m c", c=128)[:, :, 0:64],
                    AF.Copy)
            # full sums row (row 64: [vsum_h | ksum_h] per 128-col head block)
            kvs_row = single.tile([1, 1536], fp32, tag="kvs_row")
            for t in range(3):
                nc.scalar.activation(kvs_row[:, t * 512:(t + 1) * 512],
                                     kvps[t][64:65, :], AF.Copy)
            # transpose ksums to columns via K=1 matmuls; col h = ksum_h,
            # valid at partitions (h%2)*64 .. +64 (vsums transposed post-CC)
            for h in range(NUM_HEADS):
                wk_off = 128 * h + 64 - 64 * (h % 2)
                nc.tensor.matmul(sums_ps[:, h:h + 1],
                                 kvs_row[:, wk_off:wk_off + 128],
                                 ones_f[:, 0:1],
                                 start=True, stop=True, skip_group_check=True)
            sums_sb = single.tile([128, 12], fp32, tag="sums_sb")
            nc.scalar.activation(sums_sb[:], sums_ps[:], AF.Copy)

            bounce_in = dram.tile([NPAY], fp32, tag="b_in")
            bounce_out = dram.tile([NPAY], fp32, tag="b_out")
            nc.sync.dma_start(
                bounce_in[0:65 * 768].rearrange("(p f) -> p f", p=65),
                kvsb_v.rearrange("p a b -> p (a b)"))
            nc.sync.dma_start(
                bounce_in[65 * 768:NPAY].rearrange("(p f) -> p f", p=128),
                sums_sb[:])
            nc.gpsimd.collective_compute(
                "AllReduce", OP.add,
                replica_groups=[[0, 1], [2, 3], [4, 5], [6, 7]],
                ins=[bounce_in.opt()], outs=[bounce_out.opt()])

            # ---- late consts (needed from phase 2a on) ----
            wq_s = consts.tile([128, 6, C], bf16, tag="wq")
            nc.sync.dma_start(wq_s[:], wq_d.rearrange("(s p) o -> p s o", p=128))
            wo_s = consts.tile([128, 6, C], bf16, tag="wo")
            nc.sync.dma_start(wo_s[:], wo_d.rearrange("(s p) o -> p s o", p=128))
            bqp1_s = consts.tile([1, C], bf16, tag="bqp1")
            nc.sync.dma_start(bqp1_s[:], bqp1_d[None, :])
            cos_q_s = consts.tile([128, NLOC], bf16, tag="cos_q")
            nc.sync.dma_start(cos_q_s[:], cos_q_d[:])
            s2_q_s = consts.tile([128, NLOC], bf16, tag="s2_q")
            nc.sync.dma_start(s2_q_s[:], s2_q_d[:])
            eblk_s = consts.tile([NUM_HEADS, 6, 128], bf16, tag="eblk")
            nc.sync.dma_start(eblk_s[:], eblk_d.rearrange("c h p -> h c p"))

            # ================= phase 2a: q proj, elu, rope =================
            for nq in range(NF):
                nsl = slice(nq * 512, (nq + 1) * 512)
                xq_t = xin.tile([128, 6, 512], bf16, tag="xq_t")
                nc.sync.dma_start(
                    xq_t[:], xq[:, nsl].rearrange("(s p) n -> p s n", p=128))
                for oc in range(6):
                    psq = pps.tile([128, 512], fp32, tag="pp384", name="psq")
                    nc.tensor.matmul(psq[:], bqp1_s[:, oc * 128:(oc + 1) * 128],
                                     ones_row[:], start=True, stop=False)
                    for s in range(6):
                        nc.tensor.matmul(psq[:], wq_s[:, s, oc * 128:(oc + 1) * 128],
                                         xq_t[:, s, :], start=False, stop=(s == 5))
                    e_q = work.tile([128, 512], bf16, tag="e_q")
                    nc.scalar.activation(e_q[:], psq[:], AF.Exp, bias=negone[:])
                    nc.vector.scalar_tensor_tensor(
                        qpre[:, oc, nsl], e_q[:], 1.0, psq[:], OP.min, OP.max)

                # rope: A + B with B reading the 32-block-swapped q_pre
                qsw = work.tile([128, 6, 512], bf16, tag="qsw")
                for g4 in range(4):
                    sp = (g4 ^ 1) * 32
                    nc.sync.dma_start(qsw[g4 * 32:(g4 + 1) * 32, :, :],
                                      qpre[sp:sp + 32, :, nsl])
                for oc in range(6):
                    nc.vector.tensor_tensor(qrope[:, oc, nsl], qpre[:, oc, nsl],
                                            cos_q_s[:, nsl], OP.mult)
                    tmpq = work.tile([128, 512], bf16, tag="tmpq")
                    nc.vector.tensor_tensor(tmpq[:], qsw[:, oc, :], s2_q_s[:, nsl],
                                            OP.mult)
                    nc.vector.tensor_tensor(qrope[:, oc, nsl], qrope[:, oc, nsl],
                                            tmpq[:], OP.add)

            # ================= phase 2b: post-collective assembly =============
            # post-CC loads ride the scalar queue: the Act engine is idle
            # after its 2a Exps and can park at the CC-wait, while the sync
            # queue is still grinding slice-3 swap/input DMAs
            kvall_v = single.tile([65, 768], fp32, tag="kvall_v")
            nc.scalar.dma_start(kvall_v[:],
                                bounce_out[0:65 * 768].rearrange("(p f) -> p f", p=65))
            sums_all = single.tile([128, 12], fp32, tag="sums_all")
            nc.scalar.dma_start(sums_all[:],
                                bounce_out[65 * 768:NPAY].rearrange("(p f) -> p f", p=128))

            # kblock (z weights, * SFAC) via Act so it bypasses the DVE rope
            # backlog: col h=2s+t of chunk s <- ksum col h
            kb_flat = kblock.rearrange("p s h -> p (s h)")
            vb_flat = vbneg.rearrange("p s h -> p (s h)")
            for t in range(2):
                psl = slice(t * 64, (t + 1) * 64)
                nc.scalar.activation(
                    kb_flat[psl, t:t + 71:14], sums_all[psl, t:12:2],
                    AF.Copy, scale=SFAC)

            # kv2sb: block-diag [kv_2c, kv_2c+1] per chunk, * SFAC
            kva_r = kvall_v[0:64, :].rearrange("p (cc two e) -> p cc two e",
                                               two=2, e=64)
            for t in range(2):
                nc.scalar.dma_start(
                    kv2f[t * 64:(t + 1) * 64, :, t * 64:t * 64 + 64],
                    kva_r[:, :, t, :])
            nc.scalar.activation(kv2sb[:], kv2f[:], AF.Copy, scale=SFAC)

            ph1.close()
            zpool = ctx.enter_context(tc.tile_pool(name="zpool", bufs=2, space="PSUM"))
            gpool = ctx.enter_context(tc.tile_pool(name="gpool", bufs=2, space="PSUM"))
            apool = ctx.enter_context(tc.tile_pool(name="apool", bufs=2, space="PSUM"))
            opool = ctx.enter_context(tc.tile_pool(name="opool", bufs=2, space="PSUM"))

            # ========== phase 2c: z chains for all slices first ==========
            gbs = []
            for nq in range(NF):
                nsl = slice(nq * 512, (nq + 1) * 512)
                psz = zpool.tile([128, 512], fp32, tag="zp", name=f"psz{nq}")[0:NUM_HEADS, :]
                for s in range(6):
                    nc.tensor.matmul(psz[:], kblock[:, s, :], qpre[:, s, nsl],
                                     start=(s == 0), stop=(s == 5))
                gf = work.tile([NUM_HEADS, 512], fp32, tag="gf")
                nc.vector.reciprocal_approx_fast(gf[:], psz[:])  # z >= 7, eps moot
                gb = gsb.tile([NUM_HEADS, 512], bf16, tag="gb", name=f"gb{nq}")
                nc.vector.tensor_scalar_add(gb[:], gf[:], 1.0)
                nc.scalar.activation(zb[:, nsl], psz[:], AF.Copy)
                gbs.append(gb)

            # vsum columns (post-CC transposes off the trigger path), then
            # vbneg assembly and wneg = -(W~)^T : [12, 768]
            kva_row = single.tile([1, 768], fp32, tag="kva_row")
            nc.scalar.activation(kva_row[:], kvall_v[64:65, :], AF.Copy)
            vsum_ps = zpool.tile([128, 512], fp32, tag="zp", name="vsum_ps")[:, 0:12]
            for h in range(NUM_HEADS):
                wv_off = 64 * h - 64 * (h % 2)
                nc.tensor.matmul(vsum_ps[:, h:h + 1],
                                 kva_row[:, wv_off:wv_off + 128],
                                 ones_f[:, 0:1],
                                 start=True, stop=True, skip_group_check=True)
            for t in range(2):
                psl = slice(t * 64, (t + 1) * 64)
                nc.scalar.activation(
                    vb_flat[psl, t:t + 71:14], vsum_ps[psl, t:12:2],
                    AF.Copy, scale=-1.0 / N)
            for half in range(2):
                osl = slice(half * 384, (half + 1) * 384)
                psw = zpool.tile([128, 512], fp32, tag="zp", name=f"psw{half}")[0:NUM_HEADS, 0:384]
                for s in range(6):
                    nc.tensor.matmul(psw[:], vbneg[:, s, :], wo_s[:, s, osl],
                                     start=(s == 0), stop=(s == 5))
                nc.scalar.activation(wneg[:, osl], psw[:], AF.Copy)

            # ====== phase 3+4: per-slice G, q~, qkv, o_proj (pipelined) ======
            for nq in range(NF):
                nsl = slice(nq * 512, (nq + 1) * 512)
                for cc in range(6):
                    psg = gpool.tile([128, 512], fp32, tag="gp", name=f"psg{nq}_{cc}")
                    nc.tensor.matmul(psg[:], eblk_s[:, cc, :], gbs[nq][:],
                                     start=True, stop=True)
                    nc.vector.tensor_tensor(qrope[:, cc, nsl], qrope[:, cc, nsl],
                                            psg[:], OP.mult)
                    psa = apool.tile([128, 512], fp32, tag="ap", name=f"psa{nq}_{cc}")
                    nc.tensor.matmul(psa[:], kv2sb[:, cc, :], qrope[:, cc, nsl],
                                     start=True, stop=True)
                    nc.scalar.activation(res[:, cc, nsl], psa[:], AF.Copy)
                # o_proj for this n-slice
                for c2 in range(6):
                    c2sl = slice(c2 * 128, (c2 + 1) * 128)
                    pso = opool.tile([128, 512], fp32, tag="op", name=f"pso{nq}_{c2}")
                    nc.tensor.matmul(pso[:], wneg[:, c2sl], zb[:, nsl],
                                     start=True, stop=False)
                    for s in range(6):
                        nc.tensor.matmul(pso[:], wo_s[:, s, c2sl], res[:, s, nsl],
                                         start=False, stop=(s == 5))
                    osb = work.tile([128, 512], fp16, tag="osb")
                    nc.scalar.activation(osb[:], pso[:], AF.Identity,
                                         bias=bo_s[:, c2:c2 + 1])
                    nc.sync.dma_start(outT[c2sl, nsl], osb[:])

            if DEBUG:
                nc.sync.dma_start(dbg_res[:], res[:])
                nc.sync.dma_start(dbg_qrope[:], qrope[:])
                nc.sync.dma_start(dbg_zb[:], zb[:])
                nc.sync.dma_start(dbg_sums[:], sums_all[:])
                nc.sync.dma_start(dbg_kv2[:], kv2sb[:])
                nc.sync.dma_start(dbg_wneg[:], wneg[:])
                nc.sync.dma_start(dbg_kblock[:], kblock[:])

    nc.compile()
    return nc


def _get_nc():
    if "nc" not in _CACHE:
        _CACHE["nc"] = _build_nc()
    return _CACHE["nc"]


# --------------------------------------------------------------------------
# entry point
# --------------------------------------------------------------------------

def kernel(query, key, value, Wq, bq, Wk, bk, Wv, bv, Wo, bo, H, W):
    from concourse.bass_utils import run_bass_kernel_spmd

    assert int(H) == 64 and int(W) == 64
    query = np.asarray(query, np.float32)
    key = np.asarray(key, np.float32)
    value = np.asarray(value, np.float32)
    in_maps = _build_host_inputs(
        query, key, value,
        np.asarray(Wq, np.float32), np.asarray(bq, np.float32),
        np.asarray(Wk, np.float32), np.asarray(bk, np.float32),
        np.asarray(Wv, np.float32), np.asarray(bv, np.float32),
        np.asarray(Wo, np.float32), np.asarray(bo, np.float32))

    nc = _get_nc()
    kwargs = {}
    if os.environ.get("KERNEL_TRACE") == "1":
        kwargs = dict(trace=True, tmpdir=tempfile.mkdtemp(prefix="malat_"))
    r = run_bass_kernel_spmd(nc, in_maps, core_ids=list(range(NCORES)), **kwargs)
    LAST_RESULTS[0] = r

    out = np.empty((B, N, C), np.float32)
    for core in range(NCORES):
        b = core // 2
        sl = slice((core % 2) * NLOC, (core % 2 + 1) * NLOC)
        out[b, sl, :] = r.results[core]["outT"].T.astype(np.float32)
    return out
